# revision 7
# baseline (speedup 1.0000x reference)
"""Bass/Tile TRN2 kernel for nn_KnowledgeInjectionLayer.

Math:  out = x @ W_mod.T + b,  W_mod = W - P @ (P.T @ W) + P @ R_new
Low-rank identity used on-chip (avoids materializing P.T@W entirely):
    y0 = x @ W.T
    u  = y0 @ P              [tok, 8]
    v  = x @ R_new.T         [tok, 8]
    out = y0 + (v - u) @ P.T + b

Sharding: data-parallel over the 4096 tokens -> 512 tokens/core on 8 cores.
Weights replicated. Each core computes the transposed output block
outT[o, t] (o on partitions); host transposes + concatenates.

On-chip formulation (PE computes lhsT.T @ rhs, contraction on partitions):
    y0T[o_tile] = sum_ik  WT[ik, o_tile].T @ xT[ik, :]      (fp32r, N=512)
    uT = sum_ot  P[ot].T  @ y0T[ot]                          [8, 512]
    vT = sum_ik  RT[ik].T @ xT[ik]                           [8, 512]
    cT = vT - uT, augmented with a ones-row
    outT[ot] = y0T[ot] + [P|b][ot].T-transposed @ cT_aug     (bias folded in)
W and x are transposed on the PE (128x128 blocks via identity matmul).
"""

import numpy as np

import concourse.bass as bass
import concourse.mybir as mybir
import concourse.tile as tile
from concourse import bacc
from concourse.masks import make_identity

F32 = mybir.dt.float32
F32R = mybir.dt.float32r

OUT, IN, R = 4096, 4096, 8
B, S = 2, 2048
TOK = B * S          # 4096 tokens total
NCORES = 8
TPC = TOK // NCORES  # 512 tokens per core
P128 = 128
NK = IN // P128      # 32 k-tiles (contraction over IN)
NO = OUT // P128     # 32 output o-tiles
NT = TPC // P128     # 4 token tiles per core


def _build():
    nc = bacc.Bacc("TRN2", target_bir_lowering=False, debug=False)
    names = {}
    with tile.TileContext(nc) as tc:
        with tc.tile_pool(name="dram", bufs=1, space="DRAM") as dram:
            x_d = dram.tile([TPC, IN], F32, kind="ExternalInput", name="x_shard")
            w_d = dram.tile([OUT, IN], F32, kind="ExternalInput", name="w_orig")
            b_d = dram.tile([OUT], F32, kind="ExternalInput", name="b_orig")
            p_d = dram.tile([OUT, R], F32, kind="ExternalInput", name="p_mat")
            r_d = dram.tile([R, IN], F32, kind="ExternalInput", name="r_new")
            o_d = dram.tile([OUT, TPC], F32, kind="ExternalOutput", name="out_t")
            y_d = dram.tile([OUT, TPC], F32R, kind="Internal", name="y0t_spill")
            names = dict(x=x_d.name, w=w_d.name, b=b_d.name, p=p_d.name,
                         r=r_d.name, o=o_d.name)

            with tc.tile_pool(name="const", bufs=1) as const:
                ident = const.tile([P128, P128], F32)
                make_identity(nc, ident)

                # resident tensors
                xT = const.tile([P128, NK, TPC], F32R)    # x^T, 64KB/part
                paug = const.tile([P128, NO, R + 1], F32)  # [P | b] rows
                ptaug = const.tile([R + 1, NO, P128], F32R)  # ([P|b][ot]).T
                rT = const.tile([P128, NK, R], F32R)       # R_new^T blocks
                v_sb = const.tile([R, TPC], F32)
                c_sb = const.tile([R + 1, TPC], F32R)

                paug_r = const.tile([P128, NO, R], F32R)

                # ---- load P and b into the augmented [128, ot, 9] layout
                nc.sync.dma_start(
                    out=paug[:, :, 0:R],
                    in_=p_d.rearrange("(a p) r -> p a r", p=P128))
                nc.sync.dma_start(
                    out=paug[:, :, R],
                    in_=b_d.rearrange("(a p) -> p a", p=P128))
                nc.any.tensor_copy(out=paug_r, in_=paug[:, :, 0:R])

                # ---- R_new load + transpose to [i, r] blocks
                with tc.tile_pool(name="rload", bufs=1) as rp, \
                     tc.tile_pool(name="rps", bufs=2, space="PSUM") as rps:
                    r_sb = rp.tile([R, IN], F32)
                    nc.sync.dma_start(out=r_sb, in_=r_d)
                    for ik in range(NK):
                        pt = rps.tile([P128, R], F32, tag="rtp")
                        nc.tensor.transpose(
                            pt, r_sb[:, P128 * ik:P128 * (ik + 1)],
                            ident[:R, :R])
                        nc.any.tensor_copy(out=rT[:, ik, :], in_=pt)

                # ---- transpose [P|b] blocks -> ptaug
                with tc.tile_pool(name="pps", bufs=2, space="PSUM") as pps:
                    for ot in range(NO):
                        pt = pps.tile([R + 1, P128], F32, tag="ptp")
                        nc.tensor.transpose(pt, paug[:, ot, :], ident)
                        nc.any.tensor_copy(out=ptaug[:, ot, :], in_=pt)

                # ---- x load + transpose -> xT
                with tc.tile_pool(name="xn", bufs=1) as xn_pool, \
                     tc.tile_pool(name="xps", bufs=3, space="PSUM") as xps:
                    xn_tiles = []
                    for tm in range(NT):
                        xn = xn_pool.tile([P128, IN], F32, tag=f"xn{tm}")
                        nc.sync.dma_start(
                            out=xn, in_=x_d[P128 * tm:P128 * (tm + 1), :])
                        xn_tiles.append(xn)
                    for ik in range(NK):
                        ps = xps.tile([P128, TPC], F32, tag="xtp")
                        for tm in range(NT):
                            nc.tensor.transpose(
                                ps[:, P128 * tm:P128 * (tm + 1)],
                                xn_tiles[tm][:, P128 * ik:P128 * (ik + 1)],
                                ident)
                        nc.any.tensor_copy(out=xT[:, ik, :], in_=ps)

                # ---- vT = sum_ik RT[ik].T @ xT[ik]
                with tc.tile_pool(name="vps", bufs=1, space="PSUM") as vps:
                    vt = vps.tile([R, TPC], F32)
                    for ik in range(NK):
                        nc.tensor.matmul(
                            vt, rT[:, ik, :],
                            xT[:, ik, :],
                            start=(ik == 0), stop=(ik == NK - 1))
                    nc.any.tensor_copy(out=v_sb, in_=vt)

                # ---- main loop: y0T per o-tile; u accumulation; spill y0T
                with tc.tile_pool(name="wn", bufs=3) as wn_pool, \
                     tc.tile_pool(name="wt", bufs=16) as wt_pool, \
                     tc.tile_pool(name="wtps", bufs=3, space="PSUM") as wtps, \
                     tc.tile_pool(name="yps", bufs=2, space="PSUM") as yps, \
                     tc.tile_pool(name="ups", bufs=1, space="PSUM") as ups, \
                     tc.tile_pool(name="ysb", bufs=4) as ysb_pool:
                    ut = ups.tile([R, TPC], F32)
                    for ot in range(NO):
                        row = slice(P128 * ot, P128 * (ot + 1))
                        wn0 = wn_pool.tile([P128, IN // 2], F32, tag="wn")
                        wn1 = wn_pool.tile([P128, IN // 2], F32, tag="wn")
                        nc.sync.dma_start(out=wn0, in_=w_d[row, 0:IN // 2])
                        nc.sync.dma_start(out=wn1, in_=w_d[row, IN // 2:IN])
                        wts = []
                        for g in range(8):
                            wps_t = wtps.tile([P128, 512], F32, tag="wtp")
                            for s in range(4):
                                ik = 4 * g + s
                                src = wn0 if ik < NK // 2 else wn1
                                col = P128 * (ik % (NK // 2))
                                nc.tensor.transpose(
                                    wps_t[:, P128 * s:P128 * (s + 1)],
                                    src[:, col:col + P128], ident)
                            wt_t = wt_pool.tile([P128, 512], F32R, tag="wt")
                            nc.any.tensor_copy(out=wt_t, in_=wps_t)
                            wts.append(wt_t)
                        ypt = yps.tile([P128, TPC], F32, tag="y")
                        for ik in range(NK):
                            lhs = wts[ik // 4][:, P128 * (ik % 4):
                                               P128 * (ik % 4) + P128]
                            nc.tensor.matmul(
                                ypt, lhs,
                                xT[:, ik, :],
                                start=(ik == 0), stop=(ik == NK - 1))
                        y_sb = ysb_pool.tile([P128, TPC], F32R, tag="ysb")
                        nc.any.tensor_copy(out=y_sb, in_=ypt)
                        nc.tensor.matmul(
                            ut, paug_r[:, ot, :], y_sb,
                            start=(ot == 0), stop=(ot == NO - 1),
                            skip_group_check=True)
                        nc.sync.dma_start(out=y_d[row, :], in_=y_sb)

                    # cT = vT - uT ; ones row for the bias. Engines cannot
                    # address base partition 8, so fill all 9 rows with 1.0,
                    # overwrite rows 0..7, then cast-copy into the f32r tile
                    # (Memset cannot write f32r directly).
                    c_f32 = ysb_pool.tile([R + 1, TPC], F32, tag="cf")
                    nc.any.memset(c_f32, 1.0)
                    nc.vector.tensor_tensor(
                        c_f32[0:R, :], v_sb, ut, mybir.AluOpType.subtract)
                    nc.any.tensor_copy(out=c_sb, in_=c_f32)

                # ---- final: outT[ot] = y0T[ot] + ptaug[ot].T @ cT_aug
                with tc.tile_pool(name="fps", bufs=2, space="PSUM") as fps, \
                     tc.tile_pool(name="fsb", bufs=4) as fsb:
                    for ot in range(NO):
                        row = slice(P128 * ot, P128 * (ot + 1))
                        yb = fsb.tile([P128, TPC], F32R, tag="yb")
                        nc.sync.dma_start(out=yb, in_=y_d[row, :])
                        pc = fps.tile([P128, TPC], F32, tag="pc")
                        nc.tensor.matmul(
                            pc, ptaug[:, ot, :],
                            c_sb, start=True, stop=True)
                        yo = fsb.tile([P128, TPC], F32, tag="yo")
                        nc.vector.tensor_tensor(
                            yo, yb, pc, mybir.AluOpType.add)
                        nc.sync.dma_start(out=o_d[row, :], in_=yo)

    nc.compile()
    return nc, names


_CACHE = {}


def _get_program():
    if "nc" not in _CACHE:
        _CACHE["nc"], _CACHE["names"] = _build()
    return _CACHE["nc"], _CACHE["names"]


def _run_spmd(inputs, trace=False):
    from concourse.bass_utils import run_bass_kernel_spmd

    nc, names = _get_program()
    x = np.ascontiguousarray(
        np.asarray(inputs["x"], dtype=np.float32).reshape(TOK, IN))
    w = np.ascontiguousarray(np.asarray(inputs["W_orig"], dtype=np.float32))
    b = np.ascontiguousarray(np.asarray(inputs["b_orig"], dtype=np.float32))
    p = np.ascontiguousarray(np.asarray(inputs["P"], dtype=np.float32))
    r = np.ascontiguousarray(np.asarray(inputs["R_new"], dtype=np.float32))
    in_maps = []
    for c in range(NCORES):
        in_maps.append({
            names["x"]: np.ascontiguousarray(x[TPC * c:TPC * (c + 1)]),
            names["w"]: w,
            names["b"]: b,
            names["p"]: p,
            names["r"]: r,
        })
    bkr = run_bass_kernel_spmd(
        nc, in_maps, core_ids=list(range(NCORES)), trace=trace)
    out = np.empty((TOK, OUT), dtype=np.float32)
    for c in range(NCORES):
        out[TPC * c:TPC * (c + 1), :] = bkr.results[c][names["o"]].T
    return out.reshape(B, S, OUT), bkr


def kernel(**inputs):
    out, _ = _run_spmd(inputs, trace=False)
    return out


def run_and_bench(inputs):
    """Correctness output + a re-runnable jitted callable for timing.

    Replicates bass2jax.run_bass_via_pjrt but keeps one jitted function and
    device-resident inputs so repeat calls measure device execution only.
    """
    import jax
    from jax.experimental.shard_map import shard_map
    from jax.sharding import Mesh, NamedSharding, PartitionSpec

    from concourse import bass2jax

    nc, names = _get_program()
    bass2jax.install_neuronx_cc_hook()

    pname = nc.partition_id_tensor.name if nc.partition_id_tensor else None
    in_names, out_names, out_avals = [], [], []
    for alloc in nc.m.functions[0].allocations:
        if not isinstance(alloc, mybir.MemoryLocationSet):
            continue
        name = alloc.memorylocations[0].name
        if alloc.kind == "ExternalInput":
            if name != pname:
                in_names.append(name)
        elif alloc.kind == "ExternalOutput":
            out_names.append(name)
            out_avals.append(jax.core.ShapedArray(
                tuple(alloc.tensor_shape), mybir.dt.np(alloc.dtype)))
    n_params = len(in_names)
    all_in_names = in_names + out_names
    if pname is not None:
        all_in_names = all_in_names + [pname]

    def _body(*args):
        operands = list(args)
        if pname is not None:
            operands.append(bass2jax.partition_id_tensor())
        outs = bass2jax._bass_exec_p.bind(
            *operands,
            out_avals=tuple(out_avals),
            in_names=tuple(all_in_names),
            out_names=tuple(out_names),
            lowering_input_output_aliases=(),
            sim_require_finite=True,
            sim_require_nnan=True,
            nc=nc,
        )
        return tuple(outs)

    devices = jax.devices()[:NCORES]
    mesh = Mesh(np.asarray(devices), ("core",))
    nout = len(out_names)
    sharded = jax.jit(shard_map(
        _body, mesh=mesh,
        in_specs=(PartitionSpec("core"),) * (n_params + nout),
        out_specs=(PartitionSpec("core"),) * nout,
        check_rep=False))

    x = np.ascontiguousarray(
        np.asarray(inputs["x"], dtype=np.float32).reshape(TOK, IN))
    host = {
        names["x"]: x,  # already (8*512, 4096) = concat of shards
        names["w"]: np.tile(np.asarray(inputs["W_orig"], np.float32),
                            (NCORES, 1)),
        names["b"]: np.tile(np.asarray(inputs["b_orig"], np.float32), NCORES),
        names["p"]: np.tile(np.asarray(inputs["P"], np.float32), (NCORES, 1)),
        names["r"]: np.tile(np.asarray(inputs["R_new"], np.float32),
                            (NCORES, 1)),
    }
    sh = NamedSharding(mesh, PartitionSpec("core"))
    dev_args = [jax.device_put(host[n], sh) for n in in_names]
    for av in out_avals:
        z = np.zeros((NCORES * av.shape[0], *av.shape[1:]), av.dtype)
        dev_args.append(jax.device_put(z, sh))

    outs = sharded(*dev_args)
    jax.block_until_ready(outs)
    glob = np.asarray(outs[out_names.index(names["o"])])
    out = np.empty((TOK, OUT), dtype=np.float32)
    for c in range(NCORES):
        blk = glob.reshape(NCORES, OUT, TPC)[c]
        out[TPC * c:TPC * (c + 1), :] = blk.T
    result = out.reshape(B, S, OUT)

    def run_again():
        jax.block_until_ready(sharded(*dev_args))

    return result, run_again
